# revision 20
# baseline (speedup 1.0000x reference)
"""GCN layer (symmetric-normalized message passing + skip) on 8 Trainium2
NeuronCores via Bass/Tile.

    deg = bincount(src); dis = deg^-0.5 (0 where deg==0)
    out = dis_dst * ( segsum_dst( dis_src * feats[src] ) @ Wm.T ) + bm
          + feats @ Ws.T + bs

Sharding: nodes split into 8 contiguous ranges of 12500 (dst owner). Edges
partitioned by dst owner. Every core holds the full gather table in HBM.

Device algorithm per core (fully static schedule, no collectives):
  Phase 1 (deg-norm): host supplies run-boundary positions F of the sorted
    src array (integer partitioning metadata only); device computes
    deg = diff(F), dis = (deg>0)*sqrt(1/max(deg,1)), writes dis into
    column 128 of every gather-table row (so the main gather fetches each
    message row together with its source's normalizer), and builds a
    node-ordered dis for the dst-side normalization.
  Phase 2 (aggregate + linears): dst nodes in 128-blocks; per (block,
    src-subtable q) a fixed number of 128-edge tiles (int16 dma_gather
    requires 4 source sub-tables of <=32704 rows; a host-side seed search
    balances per-(block,q) loads). Per superbatch of SBL blocks, 4 batched
    dma_gathers fetch 768B rows [feat(128) | dis | pad]. Per tile, ONE fused
    DVE tensor_scalar builds onehot[e,m] = (iota==slot)*dis_src (slot=9999 on
    pad edges zeroes their column), and PE accumulates msgs.T @ onehot into
    the block's PSUM [feat, 128]. Flush: psum->SBUF rstT (= linear lhsT,
    no transpose needed), rstT@WmT into one PSUM, featsT@WsT + bias into
    another, combined as dis_dst*pm + pk (diagonal scaling commutes with the
    feature-space linear, so dst normalization applies after the matmul).
"""

import math

import numpy as np

P = 128
D = 128
NCORES = 8
ELEM = 192            # f32 per gather row = 768B (dma_gather needs %256B)
DIS_COL = 128         # dis lives at this column of each gather row
SUB = 32704           # rows per int16-indexed sub-table
NSUB = 4
PAD_SLOT = 9999.0


# ---------------------------------------------------------------- host prep

def _q_assign(src, dst, n, nloc, rng_tries=40):
    """Assign nodes to NSUB sub-tables, balancing per-(core,128-block,q) edge
    counts so T_BQ (tiles per cell) is minimal. Returns (q_of_node, T_BQ)."""
    blk = (dst % nloc) // P + (dst // nloc) * (1 << 20)  # unique cell per core-block
    _, blk_ids = np.unique(blk, return_inverse=True)
    nblk = blk_ids.max() + 1
    best = None
    for seed in range(rng_tries):
        rng = np.random.default_rng(seed)
        q = rng.integers(0, NSUB, n).astype(np.int32)
        counts = np.bincount(blk_ids * NSUB + q[src], minlength=nblk * NSUB)
        mx = counts.max()
        if best is None or mx < best[1]:
            best = (q, mx)
        if mx <= 2 * P:
            break
    q, mx = best
    return q, int(math.ceil(mx / P))


def _build_tables(feats, src, q_of_node, n):
    """Row assignment into sub-tables + the big gather table (f32).
    Returns (row_of_node [n], feats_big [NSUB*SUB, ELEM])."""
    row = np.zeros(n, np.int64)
    for qq in range(NSUB):
        nodes = np.flatnonzero(q_of_node == qq)
        assert len(nodes) <= SUB - 1, f"subtable {qq} overflow: {len(nodes)}"
        row[nodes] = qq * SUB + np.arange(len(nodes))
    feats_big = np.zeros((NSUB * SUB, ELEM), np.float32)
    feats_big[row, :D] = feats
    return row, feats_big


def _boundary_arr(sorted_vals, num_ids, rows, cols):
    """F[i] = searchsorted(sorted_vals, i) laid out [rows, cols+1] row-major
    with one overlap column. rows*cols must equal num_ids."""
    assert rows * cols == num_ids
    F = np.searchsorted(sorted_vals, np.arange(num_ids + 1)).astype(np.int32)
    out = np.zeros((rows, cols + 1), np.int32)
    for p in range(rows):
        out[p] = F[p * cols:p * cols + cols + 1]
    return out


def _wrap_idx(flat_idx):
    """dma_gather index layout: idx i at [i%16, i//16], 16-row band x8."""
    n = len(flat_idx)
    assert n % 16 == 0
    return np.tile(flat_idx.reshape(n // 16, 16).T, (8, 1)).astype(np.int16)


def _prep(feats, src, dst, wm, bm, ws, bs, ncores, sbl):
    n, d = feats.shape
    assert d == D
    nloc = n // ncores
    nchunk = int(math.ceil(nloc / P))
    nloc_pad = nchunk * P
    nsb = int(math.ceil(nchunk / sbl))
    nblk_sched = nsb * sbl

    src = np.asarray(src).astype(np.int64)
    dst = np.asarray(dst).astype(np.int64)
    feats = np.asarray(feats, dtype=np.float32)

    q_of_node, T_BQ = _q_assign(src, dst, n, nloc)
    row_of_node, feats_big = _build_tables(feats, src, q_of_node, n)

    # phase-1 metadata: run boundaries of row-sorted and node-sorted src
    tbl_rows = NSUB * SUB              # 130816 = 128 * 1022
    f_row = _boundary_arr(np.sort(row_of_node[src]), tbl_rows, P, tbl_rows // P)
    src_sorted = np.sort(src)

    T2 = nsb * NSUB * sbl * T_BQ
    nidx_op = sbl * T_BQ * P

    per_core = []
    for k in range(ncores):
        m = (dst // nloc) == k
        dl = dst[m] - k * nloc
        s = src[m]
        o = np.lexsort((s, dl))
        dl, s = dl[o], s[o]
        qq = q_of_node[s]
        lidx = (row_of_node[s] - qq * SUB).astype(np.int16)
        slot = (dl % P).astype(np.float32)
        blk = dl // P

        # bucket edges by (block, q)
        order = np.lexsort((dl, qq, blk))
        blk, qq2, lidx2, slot2 = blk[order], qq[order], lidx[order], slot[order]
        cell_of = blk * NSUB + qq2
        cap = T_BQ * P
        starts = np.searchsorted(cell_of, np.arange(nchunk * NSUB + 1),
                                 side="left")
        counts = np.diff(starts)
        assert counts.max() <= cap, f"cell overflow {counts.max()} > {cap}"
        # flat position of each edge: cell (b, q) -> op (b//sbl, q),
        # slot range [b%sbl * cap, +count)
        b_all = np.arange(nchunk * NSUB) // NSUB
        q_all = np.arange(nchunk * NSUB) % NSUB
        cell_base = ((b_all // sbl) * NSUB + q_all) * nidx_op + \
            (b_all % sbl) * cap
        within = np.arange(len(cell_of)) - starts[cell_of]
        pos = cell_base[cell_of] + within
        flat_idx = np.zeros(nsb * NSUB * nidx_op, np.int64)
        flat_idx[pos] = lidx2
        flat_slot = np.full(nsb * NSUB * nidx_op, PAD_SLOT, np.float32)
        flat_slot[pos] = slot2
        # split each (sb, q) segment into sub-ops of <=896 indices (the SWDGE
        # descriptor ring holds 64 descs/engine; 896/16+1 = 57 fits)
        nsplit = int(math.ceil(nidx_op / 896))
        assert nidx_op % (nsplit * P) == 0
        subop = nidx_op // nsplit
        g_idx = np.zeros((P, nsb * NSUB * (nidx_op // 16)), np.int16)
        for op in range(nsb * NSUB * nsplit):
            g_idx[:, op * (subop // 16):(op + 1) * (subop // 16)] = \
                _wrap_idx(flat_idx[op * subop:(op + 1) * subop])
        # g_slot[p, t] = slot of edge (tile t, row p); flat pos = t*128 + p
        g_slot = flat_slot.reshape(T2, P).T.copy()

        own = np.arange(k * nloc, k * nloc + nloc_pad)
        Fv = np.searchsorted(src_sorted, np.concatenate([own, [own[-1] + 1]]))
        f_node = np.zeros((P, nchunk + 1), np.int32)
        for p in range(P):
            f_node[p] = Fv[p * nchunk:p * nchunk + nchunk + 1]

        ft = np.zeros((P, nloc_pad), np.float32)
        ft[:, :nloc] = feats[k * nloc:(k + 1) * nloc].T

        per_core.append(dict(gidx=g_idx, gslot=g_slot, featsT=ft,
                             fnode=f_node))

    wmT = np.ascontiguousarray(np.asarray(wm, np.float32).T)
    wsT = np.ascontiguousarray(np.asarray(ws, np.float32).T)
    bm = np.asarray(bm, np.float32).reshape(1, D)
    bs = np.asarray(bs, np.float32).reshape(1, D)
    iota = np.broadcast_to(np.arange(P, dtype=np.float32), (P, P)).copy()

    cfg = dict(T_BQ=T_BQ, SBL=sbl, NSB=nsb, NLOC=nloc, NCHUNK=nchunk,
               NLOC_PAD=nloc_pad, NCORES=ncores, T2=T2, NIDX_OP=nidx_op,
               TBL_ROWS=tbl_rows, NBLK_SCHED=nblk_sched, NSPLIT=nsplit,
               SUBOP=subop)
    in_maps = []
    for k in range(ncores):
        in_maps.append({
            "gidx": per_core[k]["gidx"],
            "gslot": per_core[k]["gslot"],
            "featsT": per_core[k]["featsT"],
            "fnode": per_core[k]["fnode"],
            "frow": f_row,
            "feats_big": feats_big,
            "wmT": wmT,
            "wsT": wsT,
            "bm": bm,
            "bs": bs,
            "iota": iota,
        })
    return in_maps, cfg


# ------------------------------------------------------------- device kernel

def device_kernel(tc, outs, ins, cfg):
    import concourse.bass as bass
    import concourse.mybir as mybir

    nc = tc.nc
    f32 = mybir.dt.float32
    i32 = mybir.dt.int32
    i16 = mybir.dt.int16
    Op = mybir.AluOpType

    (out_d,) = outs
    (gidx_d, gslot_d, featsT_d, fnode_d, frow_d, feats_big_d, wmT_d, wsT_d,
     bm_d, bs_d, iota_d, dis_row_d, dis_node_d) = ins

    T_BQ, SBL, NSB = cfg["T_BQ"], cfg["SBL"], cfg["NSB"]
    NCHUNK, NLOC_PAD = cfg["NCHUNK"], cfg["NLOC_PAD"]
    T2, NIDX_OP, TBL_ROWS = cfg["T2"], cfg["NIDX_OP"], cfg["TBL_ROWS"]
    NSPLIT, SUBOP = cfg["NSPLIT"], cfg["SUBOP"]
    RCOLS = TBL_ROWS // P      # 1022
    SOPW = SUBOP // 16         # idx cols per gather sub-op

    with (
        tc.tile_pool(name="sbuf", bufs=1) as sb,
        tc.tile_pool(name="sbig", bufs=2) as sbig,
        tc.tile_pool(name="soh", bufs=6) as soh,
        tc.tile_pool(name="psag", bufs=3, space="PSUM") as psag,
        tc.tile_pool(name="pslin", bufs=2, space="PSUM") as pslin,
    ):
        # ---------------- phase 1: deg -> dis ----------------
        def dis_from_F(F_t, cols):
            degi = sb.tile([P, cols], i32, tag=f"degi{cols}")
            nc.vector.tensor_tensor(out=degi[:], in0=F_t[:, 1:cols + 1],
                                    in1=F_t[:, 0:cols], op=Op.subtract)
            degf = sb.tile([P, cols], f32, tag=f"degf{cols}")
            nc.vector.tensor_copy(out=degf[:], in_=degi[:])
            msk = sb.tile([P, cols], f32, tag=f"msk{cols}")
            nc.vector.tensor_scalar(out=msk[:], in0=degf[:], scalar1=0.0,
                                    scalar2=None, op0=Op.is_gt)
            nc.vector.tensor_scalar(out=degf[:], in0=degf[:], scalar1=1.0,
                                    scalar2=None, op0=Op.max)
            rec = sb.tile([P, cols], f32, tag=f"rec{cols}")
            nc.vector.reciprocal(out=rec[:], in_=degf[:])
            rt = sb.tile([P, cols], f32, tag=f"rt{cols}")
            nc.scalar.activation(out=rt[:], in_=rec[:],
                                 func=mybir.ActivationFunctionType.Sqrt)
            dis = sb.tile([P, cols], f32, tag=f"dis{cols}")
            nc.vector.tensor_tensor(out=dis[:], in0=rt[:], in1=msk[:],
                                    op=Op.mult)
            return dis

        frow_t = sb.tile([P, RCOLS + 1], i32)
        nc.sync.dma_start(out=frow_t[:], in_=frow_d[:])
        dis_row = dis_from_F(frow_t, RCOLS)            # [128, 1022] row-major
        nc.sync.dma_start(
            out=dis_row_d.ap().rearrange("(p c) o -> p (c o)", p=P),
            in_=dis_row[:])
        # write dis into column DIS_COL of every gather-table row
        nseg = 8
        seg = TBL_ROWS // nseg
        with nc.allow_non_contiguous_dma(reason="4B-strided dis column write"):
            for i in range(nseg):
                nc.sync.dma_start(
                    out=feats_big_d[i * seg:(i + 1) * seg,
                                    DIS_COL:DIS_COL + 1],
                    in_=dis_row_d[i * seg:(i + 1) * seg, :])

        fnode_t = sb.tile([P, NCHUNK + 1], i32)
        nc.sync.dma_start(out=fnode_t[:], in_=fnode_d[:])
        dis_nd = dis_from_F(fnode_t, NCHUNK)           # [128, 98] row-major
        nc.sync.dma_start(
            out=dis_node_d.ap().rearrange("(p c) o -> p (c o)", p=P),
            in_=dis_nd[:])
        dis_chunks = sb.tile([P, NCHUNK], f32)         # [p, c] = node c*128+p
        nc.sync.dma_start(
            out=dis_chunks[:],
            in_=dis_node_d.ap().rearrange("(c p) o -> p (c o)", p=P))

        # ---------------- phase 2 setup ----------------
        gidx = sb.tile([P, NSB * NSUB * NSPLIT * SOPW], i16)
        nc.sync.dma_start(out=gidx[:], in_=gidx_d[:])
        gslot = sb.tile([P, T2], f32)
        nc.sync.dma_start(out=gslot[:], in_=gslot_d[:])
        iota_t = sb.tile([P, P], f32)
        nc.sync.dma_start(out=iota_t[:], in_=iota_d[:])
        wmT = sb.tile([P, D], f32)
        nc.sync.dma_start(out=wmT[:], in_=wmT_d[:])
        wsT = sb.tile([P, D], f32)
        nc.sync.dma_start(out=wsT[:], in_=wsT_d[:])
        bias = sb.tile([1, D], f32)
        bs_t = sb.tile([1, D], f32)
        nc.sync.dma_start(out=bias[:], in_=bm_d[:])
        nc.sync.dma_start(out=bs_t[:], in_=bs_d[:])
        nc.vector.tensor_tensor(out=bias[:], in0=bias[:], in1=bs_t[:],
                                op=Op.add)
        ones1 = sb.tile([1, P], f32)
        nc.vector.memset(ones1[:], 1.0)
        zcol = sb.tile([1, P], f32)
        nc.vector.memset(zcol[:], 0.0)
        zrow = sb.tile([1, P], f32)
        nc.vector.memset(zrow[:], 0.0)

        MCOLS = SBL * NSUB * T_BQ * ELEM   # msgs cols per superbatch

        # ---------------- phase 2 main loop ----------------
        for sbi in range(NSB):
            msgs = sbig.tile([P, MCOLS], f32, tag="msgs")
            if sbi < 2:
                nc.vector.memset(msgs[:], 0.0)
            for q in range(NSUB):
                for so in range(NSPLIT):
                    op = (sbi * NSUB + q) * NSPLIT + so
                    scol = (q * SBL * T_BQ + so * (SUBOP // P)) * ELEM
                    ncol = (SUBOP // P) * ELEM
                    nc.gpsimd.dma_gather(
                        msgs[:, scol:scol + ncol]
                        .rearrange("p (t e) -> p t e", e=ELEM),
                        feats_big_d[q * SUB:(q + 1) * SUB, :],
                        gidx[:, op * SOPW:(op + 1) * SOPW],
                        SUBOP, SUBOP, ELEM)
            featsT_sb = sbig.tile([P, SBL * P], f32, tag="fT")
            nc.sync.dma_start(
                out=featsT_sb[:],
                in_=featsT_d[:, sbi * SBL * P:(sbi + 1) * SBL * P])

            for b_loc in range(SBL):
                b = sbi * SBL + b_loc
                if b >= NCHUNK:
                    continue
                bank = psag.tile([P, P], f32, tag="agg", space="PSUM")
                nc.tensor.matmul(out=bank[:], lhsT=zcol[:], rhs=zrow[:],
                                 start=True, stop=False)
                n_mm = NSUB * T_BQ
                mm = 0
                for q in range(NSUB):
                    for tt in range(T_BQ):
                        t = ((sbi * NSUB + q) * SBL + b_loc) * T_BQ + tt
                        c0 = ((q * SBL + b_loc) * T_BQ + tt) * ELEM
                        oh = soh.tile([P, P], f32, tag="oh")
                        nc.vector.tensor_scalar(
                            out=oh[:], in0=iota_t[:],
                            scalar1=gslot[:, t:t + 1],
                            scalar2=msgs[:, c0 + DIS_COL:c0 + DIS_COL + 1],
                            op0=Op.is_equal, op1=Op.mult)
                        mm += 1
                        nc.tensor.matmul(
                            out=bank[:],
                            lhsT=msgs[:, c0:c0 + D],
                            rhs=oh[:],
                            start=False, stop=(mm == n_mm))

                rstT = sbig.tile([P, P], f32, tag="rstT")
                nc.scalar.copy(out=rstT[:], in_=bank[:])
                pm = pslin.tile([P, D], f32, tag="pm", space="PSUM")
                nc.tensor.matmul(out=pm[:], lhsT=rstT[:], rhs=wmT[:],
                                 start=True, stop=True)
                pk = pslin.tile([P, D], f32, tag="pk", space="PSUM")
                nc.tensor.matmul(out=pk[:],
                                 lhsT=featsT_sb[:, b_loc * P:(b_loc + 1) * P],
                                 rhs=wsT[:], start=True, stop=False)
                nc.tensor.matmul(out=pk[:], lhsT=ones1[:], rhs=bias[:],
                                 start=False, stop=True)
                stage = sbig.tile([P, D], f32, tag="stage")
                nc.scalar.activation(out=stage[:], in_=pm[:],
                                     func=mybir.ActivationFunctionType.Copy,
                                     scale=dis_chunks[:, b:b + 1])
                nc.vector.tensor_tensor(out=stage[:], in0=stage[:],
                                        in1=pk[:], op=Op.add)
                nc.sync.dma_start(out=out_d[b * P:(b + 1) * P, :],
                                  in_=stage[:])


# --------------------------------------------------------------- entry point

def _build_program(cfg):
    import concourse.bacc as bacc
    import concourse.mybir as mybir
    import concourse.tile as tile

    f32 = mybir.dt.float32
    i32 = mybir.dt.int32
    i16 = mybir.dt.int16
    T2 = cfg["T2"]
    NLOC_PAD = cfg["NLOC_PAD"]
    TBL_ROWS = cfg["TBL_ROWS"]
    NCHUNK = cfg["NCHUNK"]
    ncores = cfg["NCORES"]
    OPW = cfg["SUBOP"] // 16
    NOPS = cfg["NSB"] * NSUB * cfg["NSPLIT"]

    nc = bacc.Bacc("TRN2", target_bir_lowering=False, debug=False,
                   enable_asserts=False, num_devices=ncores)

    def inp(name, shape, dt):
        return nc.dram_tensor(name, shape, dt, kind="ExternalInput").ap()

    gidx = inp("gidx", [P, NOPS * OPW], i16)
    gslot = inp("gslot", [P, T2], f32)
    featsT = inp("featsT", [P, NLOC_PAD], f32)
    fnode = inp("fnode", [P, NCHUNK + 1], i32)
    frow = inp("frow", [P, TBL_ROWS // P + 1], i32)
    feats_big = inp("feats_big", [TBL_ROWS, ELEM], f32)
    wmT = inp("wmT", [P, D], f32)
    wsT = inp("wsT", [P, D], f32)
    bm = inp("bm", [1, D], f32)
    bs = inp("bs", [1, D], f32)
    iota = inp("iota", [P, P], f32)
    out = nc.dram_tensor("out", [NLOC_PAD, D], f32, kind="ExternalOutput").ap()

    dis_row = nc.dram_tensor("dis_row", [TBL_ROWS, 1], f32)
    dis_node = nc.dram_tensor("dis_node", [NLOC_PAD, 1], f32)

    with tile.TileContext(nc) as tc:
        device_kernel(
            tc, [out],
            [gidx, gslot, featsT, fnode, frow, feats_big, wmT, wsT,
             bm, bs, iota, dis_row, dis_node],
            cfg)
    nc.compile()
    return nc


LAST_EXEC_NS = None


def kernel(feats, src, dst, linear_skip_weight, linear_skip_bias,
           linear_msg_weight, linear_msg_bias):
    global LAST_EXEC_NS
    import os

    from concourse.bass_utils import run_bass_kernel_spmd

    feats = np.asarray(feats)
    n = feats.shape[0]
    in_maps, cfg = _prep(feats, src, dst, linear_msg_weight, linear_msg_bias,
                         linear_skip_weight, linear_skip_bias, NCORES, sbl=7)
    nc = _build_program(cfg)
    trace = bool(int(os.environ.get("GCN_TRACE", "0")))
    res = run_bass_kernel_spmd(nc, in_maps, core_ids=list(range(NCORES)),
                               trace=trace)
    LAST_EXEC_NS = res.exec_time_ns
    if res.instructions_and_trace is not None:
        print("trace:", res.instructions_and_trace[1])
    nloc = cfg["NLOC"]
    out = np.empty((n, D), np.float32)
    for k in range(NCORES):
        out[k * nloc:(k + 1) * nloc] = res.results[k]["out"][:nloc]
    return out
